# revision 21
# baseline (speedup 1.0000x reference)
"""Trainium2 Bass kernel: GQA causal self-attention with ALiBi.

Problem: B=4, T=2048, C=2048, 16 Q heads / 4 KV heads, head_dim=128, fp32.

Sharding (8 cores): DP2 x TP4. Core c = (bg, g) with bg = c//4 (batches
2bg, 2bg+1), g = c%4 (KV group g = Q heads 4g..4g+3 + KV head g). The
reference's ALiBi slope is constant within a KV group (slopes[h//4]), so
each core has a single slope. Host feeds x^T per batch (transpose-free
dataflow on chip) and sums the 4 partial Wo outputs per batch.

Numerics: ALiBi decay truncates attention: key chunk kc only matters
for query chunk kc and kc+1 (distance >= 129 keys has relative weight
< e^-32 even at the smallest slope 0.25), so each 128-query column
attends to exactly 2 key chunks (prior + diagonal); truncation error
~1e-6 on the final output.

Softmax trick: for query column c the true exponent is
s + sigma*(k_glob - q_glob). Any per-query additive constant cancels
between the numerator and denominator of softmax, so we compute
  E = s + sigma*(k_glob - 128c - 64)
instead, which is a pure per-PARTITION bias per key chunk half
(prior: sigma*(k-192), diag: sigma*(k-64)) riding the ACT exp's bias
AP -- no mask matmul at all. The dropped per-query factor
exp(sigma*(q_loc-64)) scales numerator and denominator identically.
Range: |E| <= max(s) + 64*sigma ~ 51 nats, safely inside fp32/bf16;
far keys underflow exp to 0 exactly where ALiBi has already zeroed
them. The causal mask within the diagonal chunk is the only 2D term
left: a 0/1 lower-triangular DVE multiply.

v7 three-stream software pipeline. The attention of a block needs
~25us of ACT+DVE work (exps, causal mask, denominator adds, evacs) but
only ~5us of PE work, while the projections are the opposite; running
them as separate phases made ACT/DVE the bottleneck during attention
and left them idle during projections (observed as ~3us PE stalls per
block + HAM clock re-throttle after each). Phase p interleaves, slot
by slot:
  - projections of block p         (PE-dense filler)
  - attention of block p-1         (ACT/DVE-heavy, PE-light)
  - output projection of block p-2 (PE-dense, evac deferred one slot)
so every engine sees a steady mix. PV matmuls lag their scores/exp/
mask chain by two slots; each head's softmax tail (all-ones denominator
matmul -> DVE fast reciprocal -> DVE multiply) is deferred into the
next head's slot. The K^T/V rings grow to 9 slots so projections of
block p can land while attention of p-1 still reads its window.

The wq weight DMA is split per 128-contraction chunk and interleaved
with block 0's x strip so the first Q matmul starts ~10us in.
"""

import math
from contextlib import ExitStack

import ml_dtypes
import numpy as np

import concourse.bass as bass
import concourse.mybir as mybir
import concourse.tile as tile
from concourse import bacc
from concourse.bass_utils import run_bass_kernel_spmd

B, T, C = 4, 2048, 2048
HD = 128          # head dim
HPC = 4           # Q heads per core
QB = 512          # query block (projection tile free dim)
KC = 128          # key chunk / query column
NQB = T // QB     # 4
NCC = C // 128    # 16 contraction chunks for projections
NBLK = 2 * NQB    # 8 blocks: (b, tb) = (blk // 4, blk % 4)
RING = 9

F32 = mybir.dt.float32
BF16 = mybir.dt.bfloat16
EXP = mybir.ActivationFunctionType.Exp

_CACHE = {}


def build_kernel():
    nc = bacc.Bacc(
        "TRN2",
        target_bir_lowering=False,
        debug=False,
        enable_asserts=False,
        num_devices=8,
    )
    xT2 = nc.dram_tensor("xT2", [2, C, T], BF16, kind="ExternalInput").ap()
    wq_d = nc.dram_tensor("wq", [C, HPC * HD], BF16, kind="ExternalInput").ap()
    wk_d = nc.dram_tensor("wk", [C, HD], BF16, kind="ExternalInput").ap()
    wv_d = nc.dram_tensor("wv", [C, HD], BF16, kind="ExternalInput").ap()
    wo_d = nc.dram_tensor("wo", [HPC * HD, C], BF16, kind="ExternalInput").ap()
    bias_d = nc.dram_tensor("biask", [KC, 2], F32, kind="ExternalInput").ap()
    sig_d = nc.dram_tensor("sigmas", [KC, 1], F32, kind="ExternalInput").ap()
    tri_d = nc.dram_tensor("trimask", [KC, KC], BF16, kind="ExternalInput").ap()
    on_d = nc.dram_tensor("onesc", [128, 128], BF16, kind="ExternalInput").ap()
    outT = nc.dram_tensor("outT", [2, C, T], BF16, kind="ExternalOutput").ap()

    wq_r = wq_d.rearrange("(cc p) d -> p cc d", p=128)

    with ExitStack() as ctx:
        tc = ctx.enter_context(tile.TileContext(nc))
        ctx.enter_context(
            nc.allow_low_precision(reason="bf16 inputs, fp32 accumulate")
        )

        consts = ctx.enter_context(tc.tile_pool(name="consts", bufs=1))
        xpool = ctx.enter_context(tc.tile_pool(name="xpool", bufs=34))
        kvpool = ctx.enter_context(tc.tile_pool(name="kvpool", bufs=1))
        qpool = ctx.enter_context(tc.tile_pool(name="qpool", bufs=3))
        ypool = ctx.enter_context(tc.tile_pool(name="ypool", bufs=2))
        apool = ctx.enter_context(tc.tile_pool(name="apool", bufs=4))
        ppool = ctx.enter_context(tc.tile_pool(name="ppool", bufs=10))
        opool = ctx.enter_context(tc.tile_pool(name="opool", bufs=8))
        rpool = ctx.enter_context(tc.tile_pool(name="rpool", bufs=3))

        ps_acc = ctx.enter_context(tc.tile_pool(name="ps_acc", bufs=3, space="PSUM"))
        ps_s = ctx.enter_context(tc.tile_pool(name="ps_s", bufs=2, space="PSUM"))
        ps_y = ctx.enter_context(tc.tile_pool(name="ps_y", bufs=2, space="PSUM"))
        ps_d = ctx.enter_context(tc.tile_pool(name="ps_d", bufs=1, space="PSUM"))

        # wq arrives per 128-chunk, interleaved with block 0's x strip:
        # the first Q matmul gates only on chunk 0 of each.
        wq_sb = consts.tile([128, NCC, HPC * HD], BF16)
        xts0 = []
        for cc in range(NCC):
            nc.sync.dma_start(wq_sb[:, cc, :], wq_r[:, cc, :])
            xt = xpool.tile([128, QB], BF16, tag="x")
            # x strip on the (startup-idle) ACT DMA queue: descriptor
            # setup runs in parallel with the wq chunks on the SP queue
            nc.scalar.dma_start(xt, xT2[0, cc * 128:(cc + 1) * 128, 0:QB])
            xts0.append(xt)

        wk_sb = consts.tile([128, NCC, HD], BF16)
        nc.sync.dma_start(wk_sb, wk_d.rearrange("(cc p) d -> p cc d", p=128))
        wv_sb = consts.tile([128, NCC, HD], BF16)
        nc.sync.dma_start(wv_sb, wv_d.rearrange("(cc p) d -> p cc d", p=128))
        bias_sb = consts.tile([128, 2], F32)
        nc.sync.dma_start(bias_sb, bias_d)
        sig_sb = consts.tile([128, 1], F32)
        nc.sync.dma_start(sig_sb, sig_d)
        tri_sb = consts.tile([128, KC], BF16)
        nc.sync.dma_start(tri_sb, tri_d)
        ones = consts.tile([128, 128], BF16)
        nc.sync.dma_start(ones, on_d)
        wo_sb = consts.tile([128, HPC, C], BF16)
        nc.sync.dma_start(wo_sb, wo_d.rearrange("(hc p) c -> p hc c", p=128))

        kt_ring = kvpool.tile([128, RING, KC], BF16, tag="kt")
        v_ring = kvpool.tile([128, RING, HD], BF16, tag="v")

        qts = {}   # blk -> qT_sb
        ys = {}    # blk -> y_sb

        # ---------- stream builders: each returns a list of closures ----------

        def proj_stream(blk):
            """Q/K/V projections of block `blk` as ~14 PE-dense slices."""
            b, tb = blk // 4, blk % 4
            t0 = tb * QB
            if blk == 0:
                xts = xts0
            else:
                xts = []
                for cc in range(NCC):
                    xt = xpool.tile([128, QB], BF16, tag="x")
                    nc.sync.dma_start(
                        xt, xT2[b, cc * 128:(cc + 1) * 128, t0:t0 + QB]
                    )
                    xts.append(xt)
            qT_sb = qpool.tile([128, HPC, QB], BF16)
            qts[blk] = qT_sb
            work = []

            def q_head(h, lo, hi, ps_box):
                def emit():
                    if lo == 0:
                        ps_box.append(ps_acc.tile([128, QB], F32, tag="acc", name="ps"))
                    ps = ps_box[0]
                    for cc in range(lo, hi):
                        nc.tensor.matmul(
                            ps,
                            lhsT=wq_sb[:, cc, h * HD:(h + 1) * HD],
                            rhs=xts[cc],
                            start=(cc == 0), stop=(cc == NCC - 1),
                        )
                    if hi == NCC:
                        nc.vector.tensor_copy(qT_sb[:, h, :], ps)
                return emit

            for h in range(HPC):
                box = []
                work.append(q_head(h, 0, 8, box))
                work.append(q_head(h, 8, NCC, box))

            def k_part(lo, hi, ps_box):
                def emit():
                    if lo == 0:
                        ps_box.append(ps_acc.tile([128, QB], F32, tag="acc", name="ps"))
                    ps = ps_box[0]
                    for cc in range(lo, hi):
                        nc.tensor.matmul(
                            ps, lhsT=wk_sb[:, cc, :], rhs=xts[cc],
                            start=(cc == 0), stop=(cc == NCC - 1),
                        )
                    if hi == NCC:
                        for kc in range(4):
                            nc.scalar.copy(
                                kt_ring[:, (blk * 4 + kc) % RING, :],
                                ps[:, kc * KC:(kc + 1) * KC],
                            )
                return emit
            kbox = []
            work.append(k_part(0, 8, kbox))
            work.append(k_part(8, NCC, kbox))

            def v_part(v4):
                def emit():
                    ps = ps_acc.tile([128, KC], F32, tag="acc")
                    for cc in range(NCC):
                        nc.tensor.matmul(
                            ps,
                            lhsT=xts[cc][:, v4 * KC:(v4 + 1) * KC],
                            rhs=wv_sb[:, cc, :],
                            start=(cc == 0), stop=(cc == NCC - 1),
                        )
                    nc.vector.tensor_copy(
                        v_ring[:, (blk * 4 + v4) % RING, :], ps
                    )
                return emit
            for v4 in range(4):
                work.append(v_part(v4))
            return work

        def attn_stream(blk):
            """Attention of block `blk`: 16 (head, qcol) slots; PV lags
            its chain by 2 slots; per-head softmax tail deferred into the
            next head's first slot (last head's tail is an extra
            closure)."""
            b, tb = blk // 4, blk % 4
            qT_sb = qts.pop(blk)
            y_sb = ypool.tile([128, HPC, QB], BF16)
            ys[blk] = y_sb
            work = []
            state = {"fin": None}

            def slot(h, c, yb, ab, pvs):
                kb = 4 * blk + c          # global key-chunk index (ring slot)
                has_prior = 4 * tb + c > 0  # sequence-local: batch restart

                def emit():
                    if c == 0:
                        yb.append(ps_y.tile([128, QB], F32, tag="y", name="y_ps"))
                        ab.append(apool.tile([128, QB], BF16, tag="a", name="acc"))
                    y_ps, acc = yb[0], ab[0]
                    qsl = qT_sb[:, h, c * KC:(c + 1) * KC]
                    s_ps = ps_s.tile([128, QB], F32, tag="s")
                    if has_prior:
                        nc.tensor.matmul(
                            s_ps[:, 0:KC],
                            lhsT=kt_ring[:, (kb - 1) % RING, :],
                            rhs=qsl, start=True, stop=False,
                        )
                    nc.tensor.matmul(
                        s_ps[:, KC:2 * KC],
                        lhsT=kt_ring[:, kb % RING, :],
                        rhs=qsl, start=not has_prior, stop=True,
                    )
                    if c == 0 and state["fin"] is not None:
                        state["fin"]()
                        state["fin"] = None
                    if c >= 2:
                        pvs[c - 2]()
                    pT = ppool.tile([128, 2 * KC], BF16, tag="p")
                    if has_prior:
                        nc.scalar.activation(
                            pT[:, 0:KC], s_ps[:, 0:KC], EXP,
                            bias=bias_sb[:, 0:1], scale=sig_sb[:, 0:1],
                        )
                    nc.scalar.activation(
                        pT[:, KC:2 * KC], s_ps[:, KC:2 * KC], EXP,
                        bias=bias_sb[:, 1:2], scale=sig_sb[:, 0:1],
                    )
                    nc.vector.tensor_mul(
                        pT[:, KC:2 * KC], pT[:, KC:2 * KC], tri_sb
                    )
                    asl = acc[:, c * KC:(c + 1) * KC]
                    if has_prior:
                        nc.vector.tensor_add(
                            asl, pT[:, 0:KC], pT[:, KC:2 * KC]
                        )
                    else:
                        nc.vector.tensor_copy(asl, pT[:, KC:2 * KC])

                    def pv():
                        ysl = y_ps[:, c * KC:(c + 1) * KC]
                        if has_prior:
                            nc.tensor.matmul(
                                ysl, lhsT=v_ring[:, (kb - 1) % RING, :],
                                rhs=pT[:, 0:KC],
                                start=(c == 0), stop=False,
                            )
                        nc.tensor.matmul(
                            ysl, lhsT=v_ring[:, kb % RING, :],
                            rhs=pT[:, KC:2 * KC],
                            start=(c == 0 and not has_prior),
                            stop=(c == 3),
                        )
                    pvs.append(pv)
                    if c == 3:
                        def fin():
                            pvs[2]()
                            pvs[3]()
                            den_ps = ps_d.tile([128, QB], F32, tag="d")
                            nc.tensor.matmul(den_ps, lhsT=ones, rhs=acc)
                            rec = rpool.tile([128, QB], F32, tag="rec")
                            nc.vector.reciprocal_approx_fast(rec, den_ps)
                            nc.vector.tensor_mul(y_sb[:, h, :], y_ps, rec)
                        state["fin"] = fin
                return emit

            for h in range(HPC):
                yb, ab, pvs = [], [], []
                for c in range(4):
                    work.append(slot(h, c, yb, ab, pvs))

            def flush():
                state["fin"]()
                state["fin"] = None
            work.append(flush)
            return work

        def oproj_stream(blk):
            """Output projection of block `blk` as 16 chunk closures; each
            chunk's PSUM->SBUF evac + store is deferred into the next
            chunk's closure so it never front-runs the exp chain on
            ACT/DVE."""
            b, tb = blk // 4, blk % 4
            t0 = tb * QB
            y_sb = ys.pop(blk)
            work = []
            state = {"evac": None}

            def chunk(co):
                def emit():
                    o_ps = ps_acc.tile([128, QB], F32, tag="acc")
                    for hc in range(HPC):
                        nc.tensor.matmul(
                            o_ps,
                            lhsT=wo_sb[:, hc, co * 128:(co + 1) * 128],
                            rhs=y_sb[:, hc, :],
                            start=(hc == 0), stop=(hc == HPC - 1),
                        )
                    if state["evac"] is not None:
                        state["evac"]()

                    def evac():
                        o_sb = opool.tile([128, QB], BF16, tag="o")
                        if co % 2 == 0:
                            nc.scalar.copy(o_sb, o_ps)
                        else:
                            nc.vector.tensor_copy(o_sb, o_ps)
                        nc.sync.dma_start(
                            outT[b, co * 128:(co + 1) * 128, t0:t0 + QB],
                            o_sb,
                        )
                    state["evac"] = evac
                return emit
            for co in range(16):
                work.append(chunk(co))

            def last_evac():
                state["evac"]()
            work.append(last_evac)
            return work

        # ---------- interleave the three streams phase by phase ----------
        for p in range(NBLK + 2):
            streams = []
            if p < NBLK:
                streams.append(proj_stream(p))
            if 1 <= p <= NBLK:
                streams.append(attn_stream(p - 1))
            if 2 <= p <= NBLK + 1:
                streams.append(oproj_stream(p - 2))
            n = max(len(s) for s in streams)
            for i in range(n):
                for s in streams:
                    if i < len(s):
                        s[i]()

    nc.compile()
    return nc


def kernel(x, Wq, Wk, Wv, Wo):
    import os
    import time

    dbg = os.environ.get("KERNEL_DEBUG") == "1"
    t0 = time.time()

    def tick(msg):
        nonlocal t0
        if dbg:
            print(f"[kernel] {msg}: {time.time() - t0:.2f}s", flush=True)
        t0 = time.time()

    x = np.ascontiguousarray(np.asarray(x, np.float32))
    Wq = np.ascontiguousarray(np.asarray(Wq, np.float32))
    Wk = np.ascontiguousarray(np.asarray(Wk, np.float32))
    Wv = np.ascontiguousarray(np.asarray(Wv, np.float32))
    Wo = np.ascontiguousarray(np.asarray(Wo, np.float32))

    tick("input prep")
    if "nc" not in _CACHE:
        _CACHE["nc"] = build_kernel()
        tick("build_kernel")
    nc = _CACHE["nc"]

    s = 1.0 / math.sqrt(HD)
    slopes = [2.0 ** -0.5, 0.5, 2.0 ** -1.5, 0.25]
    BF = ml_dtypes.bfloat16

    k_idx = np.arange(KC, dtype=np.float32)
    trimask = (k_idx[:, None] <= k_idx[None, :]).astype(BF)

    in_maps = []
    for c in range(8):
        bg, g = c // 4, c % 4
        xT2 = np.stack(
            [np.ascontiguousarray(x[2 * bg + i].T) for i in range(2)]
        )
        sg = slopes[g]
        biask = np.stack(
            [sg * (k_idx - 192.0), sg * (k_idx - 64.0)], axis=1
        ).astype(np.float32)
        in_maps.append({
            "xT2": xT2.astype(BF),
            "wq": (Wq[:, g * 512:(g + 1) * 512] * (s / sg)).astype(BF),
            "wk": Wk[:, g * HD:(g + 1) * HD].astype(BF),
            "wv": Wv[:, g * HD:(g + 1) * HD].astype(BF),
            "wo": Wo[g * 512:(g + 1) * 512, :].astype(BF),
            "biask": biask,
            "sigmas": np.full((KC, 1), sg, np.float32),
            "trimask": trimask,
            "onesc": np.ones((128, 128), BF),
        })

    tick("in_maps prep")
    res = run_bass_kernel_spmd(nc, in_maps, core_ids=list(range(8)))
    tick("device run")
    out = np.zeros((B, T, C), np.float32)
    for c in range(8):
        bg, g = c // 4, c % 4
        oT = np.asarray(res.results[c]["outT"], np.float32)
        for i in range(2):
            out[2 * bg + i] += oT[i].T
    tick("gather")
    return out
